# revision 17
# baseline (speedup 1.0000x reference)
"""Grouped GEMM (MoE routing) kernel for 8 Trainium2 NeuronCores.

Computation: for expert e, rows seg_indptr[e]:seg_indptr[e+1] of a[M,K] are
multiplied by b[e]^T (b is [E,N,K]), then scaled per-token (scale_a) and
per-expert (scale_b).

Strategy: 2D grid of 4 N-groups x 2 K-halves across the 8 cores. Core c
handles N columns [g*1408, (g+1)*1408) (g = c//2) for K rows
[h*1024, (h+1)*1024) (h = c%2), over ALL M token rows; the host sums the two
K-half partials per N-group. 1408 = 11*128 exactly, so every stationary
weight chunk is full 128 wide (no PE column waste), and K=1024 = 8*128.

The per-expert segment structure (from seg_indptr, known on host at call
time) is baked into a single SPMD program shared by all 8 cores; per-core
differences are input *values* only. Scales are folded into `a` on the host
(row scaling commutes with the GEMM).

All device IO is bfloat16 (matmul runs at the same 1 row/cycle as fp32r but
halves DMA + SBUF traffic, which otherwise steals SBUF ports from the
matmul moving-operand stream); PSUM accumulation stays fp32 and the two
K-half partials are summed in fp32 on the host, so end-to-end error is just
input/output rounding (~3e-3 rel).

Host-packed DMA-friendly tiled layouts (partition line = one contiguous
descriptor; >=2KB contiguous per partition keeps the DMA queues at line
rate):
  a [NCH, 128, 8, 512]     a[ci, p, kc, m]    = a_scaled[m0_ci+m, h*1024+kc*128+p]
  w [E, 128, 11, 8, 128]   w[e, p, ch, kc, n] = b[e, g*1408+ch*128+n, h*1024+kc*128+p]
  o [NCH, 128, 11, 512]    o[ci, p, ch, m]    = out_partial[m0_ci+m, g*1408+ch*128+p]

Schedule notes (from NTFF traces): experts are processed in
descending-size order so the first weight load overlaps a long compute
stretch; whole-expert weight loads ride the sync ring, activation chunks
the gpsimd ring, stores alternate scalar/gpsimd, PSUM->SBUF bf16 casts on
vector. The pipeline fill is latency-bound on cold DMA queues (~3-4us
each), so the first activation chunk goes out per kc-piece and the first
expert's weights stream per 128-column chunk across two rings.
Steady-state matmul issue rate measured at the 1 row/cycle @2.4GHz ideal;
fp8 DoubleRow measured at only 2x bf16 on hw (cost model claims 4x), so
an error-compensated 3-matmul fp8 scheme (1.5x cycles) loses to bf16.
"""

import sys

import numpy as np
import ml_dtypes

_TRN = "/opt/trn_rl_repo"
if _TRN not in sys.path:
    sys.path.insert(0, _TRN)

M, K, N, E = 16384, 2048, 5632, 8
NCORES = 8
NGROUPS = 4  # N split
NSLICE = N // NGROUPS  # 1408 = 11 * 128
NCH_N = NSLICE // 128  # 11
KHALF = K // 2  # 1024
KC = KHALF // 128  # 8
P = 128
MCHUNK = 512

BF16 = ml_dtypes.bfloat16

_cache: dict = {}


def _chunks_of(segs):
    """[(m0, mjw, mjw_mm)] for all experts' m-chunks + [(expert, count)].

    Experts are processed in descending-size order: the first expert's
    weight load then overlaps a long compute stretch (no startup
    starvation) and the tiny experts land at the tail where their stores
    are nearly free. Chunk sizes are balanced per expert (all <= 512,
    near-equal) so there is no padded-to-512 tail. mjw_mm (the matmul
    moving size) just rounds odd sizes up to even.
    """
    order = sorted(
        (e for e in range(len(segs)) if segs[e][1] > 0),
        key=lambda e: -segs[e][1],
    )
    chunks = []
    elist = []
    for e in order:
        m_start, m_len = segs[e]
        cnt = -(-m_len // MCHUNK)
        s = 2 * (-(-m_len // (2 * cnt)))  # even, balanced
        sizes = [s] * (cnt - 1) + [m_len - s * (cnt - 1)]
        m0 = m_start
        for mjw in sizes:
            mjw_mm = mjw + (mjw & 1)
            chunks.append((m0, mjw, mjw_mm))
            m0 += mjw
        elist.append((e, cnt))
    return chunks, elist


def _build_program(segs):
    from concourse import bacc
    import concourse.mybir as mybir
    import concourse.tile as tile

    f32 = mybir.dt.float32
    bf16 = mybir.dt.bfloat16

    chunks, elist = _chunks_of(segs)
    nch = len(chunks)

    nc = bacc.Bacc(name="grouped_gemm")
    a_p = nc.declare_dram_parameter("a", [nch, P, KC, MCHUNK], bf16, isOutput=False)
    w_p = nc.declare_dram_parameter("w", [E, P, NCH_N, KC, P], bf16, isOutput=False)
    o_p = nc.declare_dram_parameter("o", [nch, P, NCH_N, MCHUNK], bf16, isOutput=True)

    with (
        tile.TileContext(nc) as tc,
        tc.tile_pool(name="wp", bufs=3) as wp,
        tc.tile_pool(name="apool", bufs=4) as apool,
        tc.tile_pool(name="spool", bufs=3) as spool,
        tc.tile_pool(name="pspool", bufs=8, space="PSUM") as pspool,
    ):
        ci = 0
        first = True
        for e, count in elist:
            w_t = wp.tile([P, NCH_N, KC, P], bf16, tag="w")
            if first:
                # Pipeline fill: the first expert's weights stream per
                # 128-column chunk across the sync and scalar rings (scalar
                # carries nothing else at t=0), so the ch=0 accumulation
                # group starts after ~0.25 MB instead of ~3 MB.
                for ch in range(NCH_N):
                    ring = nc.sync if ch % 2 == 0 else nc.scalar
                    ring.dma_start(w_t[:, ch], w_p[e, :, ch])
            else:
                nc.sync.dma_start(w_t[:], w_p[e])
            for _ in range(count):
                _, mjw, mjw_mm = chunks[ci]
                a_t = apool.tile([P, KC, MCHUNK], bf16, tag="a")
                if first:
                    # First chunk's a per kc-piece: the (ch0, kc0) matmul
                    # needs only the first piece.
                    for kc in range(KC):
                        nc.gpsimd.dma_start(
                            a_t[:, kc, :mjw_mm], a_p[ci, :, kc, :mjw_mm]
                        )
                    first = False
                else:
                    nc.gpsimd.dma_start(a_t[:], a_p[ci])
                st = spool.tile([P, NCH_N, MCHUNK], bf16, tag="st")
                for ch in range(NCH_N):
                    ps = pspool.tile([P, MCHUNK], f32, tag="ps")
                    for kc in range(KC):
                        nc.tensor.matmul(
                            ps[:, :mjw_mm],
                            w_t[:, ch, kc, :],
                            a_t[:, kc, :mjw_mm],
                            start=(kc == 0),
                            stop=(kc == KC - 1),
                        )
                    nc.vector.tensor_copy(st[:, ch, :mjw], ps[:, :mjw])
                    if mjw > P:
                        s_ring = nc.scalar if ch % 2 == 0 else nc.gpsimd
                        s_ring.dma_start(o_p[ci, :, ch, :mjw], st[:, ch, :mjw])
                if mjw <= P:
                    # Tiny chunk: one merged store instead of 11 descriptor
                    # issues (~600ns each) draining at the tail.
                    nc.scalar.dma_start(o_p[ci, :, :, :mjw], st[:, :, :mjw])
                ci += 1

    nc.finalize()
    return nc


def _get_program(segs):
    nc = _cache.get(segs)
    if nc is None:
        nc = _build_program(segs)
        _cache[segs] = nc
    return nc


def kernel(a, b, scale_a, scale_b, seg_indptr, batch_size, _want_trace=False):
    from concourse.bass_utils import run_bass_kernel_spmd

    a = np.asarray(a, dtype=np.float32)
    b = np.asarray(b, dtype=np.float32)
    scale_a = np.asarray(scale_a, dtype=np.float32).reshape(M, 1)
    scale_b = np.asarray(scale_b, dtype=np.float32).reshape(E, 1)
    seg = np.asarray(seg_indptr).astype(np.int64)

    segs = []
    row_scale = np.empty((M, 1), dtype=np.float32)
    for e in range(E):
        s, t = int(seg[e]), int(seg[e + 1])
        s, t = max(0, min(s, M)), max(0, min(t, M))
        segs.append((s, max(0, t - s)))
        if t > s:
            row_scale[s:t] = scale_b[e, 0]
    segs = tuple(segs)
    row_scale *= scale_a

    chunks, _counts = _chunks_of(segs)
    nch = len(chunks)
    nc = _get_program(segs)

    a_scaled = (a * row_scale).astype(BF16)  # [M, K]
    # Pack a chunks per K-half: a_pk[h][ci, p, kc, m]
    a_pk = [np.zeros((nch, P, KC, MCHUNK), dtype=BF16) for _ in range(2)]
    for ci, (m0, mjw, _mm) in enumerate(chunks):
        blk = a_scaled[m0 : m0 + mjw]  # [mjw, K]
        # [mjw, 2, 8, 128] -> (h, p, kc, m)
        blk4 = blk.reshape(mjw, 2, KC, P).transpose(1, 3, 2, 0)
        a_pk[0][ci, :, :, :mjw] = blk4[0]
        a_pk[1][ci, :, :, :mjw] = blk4[1]

    # Pack weights per core: w[e, p, kc, n] = b[e, g*1408+n, h*1024+kc*128+p]
    b16 = b.astype(BF16)
    in_maps = []
    for c in range(NCORES):
        g, h = c // 2, c % 2
        bw = b16[:, g * NSLICE : (g + 1) * NSLICE, h * KHALF : (h + 1) * KHALF]
        # [E, (ch,n128), (kc,p)] -> [E, p, ch, kc, n128]
        w_c = np.ascontiguousarray(
            bw.reshape(E, NCH_N, P, KC, P).transpose(0, 4, 1, 3, 2)
        )
        in_maps.append({"a": a_pk[h], "w": w_c})

    res = run_bass_kernel_spmd(
        nc, in_maps, list(range(NCORES)), trace=_want_trace
    )

    out = np.empty((M, N), dtype=np.float32)
    for g in range(NGROUPS):
        o_sum = res.results[2 * g]["o"].astype(np.float32) + res.results[
            2 * g + 1
        ]["o"].astype(np.float32)
        for ci, (m0, mjw, _mm) in enumerate(chunks):
            # [p, ch, m] -> [m, ch, p] -> [mjw, 1408]
            out[m0 : m0 + mjw, g * NSLICE : (g + 1) * NSLICE] = (
                o_sum[ci, :, :, :mjw].transpose(2, 1, 0).reshape(mjw, NSLICE)
            )
    if _want_trace:
        return out, res
    return out


# revision 18
# speedup vs baseline: 1.0029x; 1.0029x over previous
"""Grouped GEMM (MoE routing) kernel for 8 Trainium2 NeuronCores.

Computation: for expert e, rows seg_indptr[e]:seg_indptr[e+1] of a[M,K] are
multiplied by b[e]^T (b is [E,N,K]), then scaled per-token (scale_a) and
per-expert (scale_b).

Strategy: 2D grid of 4 N-groups x 2 K-halves across the 8 cores. Core c
handles N columns [g*1408, (g+1)*1408) (g = c//2) for K rows
[h*1024, (h+1)*1024) (h = c%2), over ALL M token rows; the host sums the two
K-half partials per N-group. 1408 = 11*128 exactly, so every stationary
weight chunk is full 128 wide (no PE column waste), and K=1024 = 8*128.

The per-expert segment structure (from seg_indptr, known on host at call
time) is baked into a single SPMD program shared by all 8 cores; per-core
differences are input *values* only. Scales are folded into `a` on the host
(row scaling commutes with the GEMM).

All device IO is bfloat16 (matmul runs at the same 1 row/cycle as fp32r but
halves DMA + SBUF traffic, which otherwise steals SBUF ports from the
matmul moving-operand stream); PSUM accumulation stays fp32 and the two
K-half partials are summed in fp32 on the host, so end-to-end error is just
input/output rounding (~3e-3 rel).

Host-packed DMA-friendly tiled layouts (partition line = one contiguous
descriptor; >=2KB contiguous per partition keeps the DMA queues at line
rate):
  a [NCH, 128, 8, 512]     a[ci, p, kc, m]    = a_scaled[m0_ci+m, h*1024+kc*128+p]
  w [E, 128, 11, 8, 128]   w[e, p, ch, kc, n] = b[e, g*1408+ch*128+n, h*1024+kc*128+p]
  o [NCH, 128, 11, 512]    o[ci, p, ch, m]    = out_partial[m0_ci+m, g*1408+ch*128+p]

Schedule notes (from NTFF traces): experts are processed in
descending-size order so the first weight load overlaps a long compute
stretch; whole-expert weight loads ride the sync ring, activation chunks
the gpsimd ring, stores alternate scalar/gpsimd, PSUM->SBUF bf16 casts on
vector. The pipeline fill is latency-bound on cold DMA queues (~3-4us
each), so the first activation chunk goes out per kc-piece and the first
expert's weights stream per 128-column chunk across two rings.
Steady-state matmul issue rate measured at the 1 row/cycle @2.4GHz ideal;
fp8 DoubleRow measured at only 2x bf16 on hw (cost model claims 4x), so
an error-compensated 3-matmul fp8 scheme (1.5x cycles) loses to bf16.
"""

import sys

import numpy as np
import ml_dtypes

_TRN = "/opt/trn_rl_repo"
if _TRN not in sys.path:
    sys.path.insert(0, _TRN)

M, K, N, E = 16384, 2048, 5632, 8
NCORES = 8
NGROUPS = 4  # N split
NSLICE = N // NGROUPS  # 1408 = 11 * 128
NCH_N = NSLICE // 128  # 11
KHALF = K // 2  # 1024
KC = KHALF // 128  # 8
P = 128
MCHUNK = 512

BF16 = ml_dtypes.bfloat16

_cache: dict = {}


def _chunks_of(segs):
    """[(m0, mjw, mjw_mm)] for all experts' m-chunks + [(expert, count)].

    Experts are processed in descending-size order: the first expert's
    weight load then overlaps a long compute stretch (no startup
    starvation) and the tiny experts land at the tail where their stores
    are nearly free. Chunk sizes are balanced per expert (all <= 512,
    near-equal) so there is no padded-to-512 tail. mjw_mm (the matmul
    moving size) just rounds odd sizes up to even.
    """
    order = sorted(
        (e for e in range(len(segs)) if segs[e][1] > 0),
        key=lambda e: -segs[e][1],
    )
    chunks = []
    elist = []
    for e in order:
        m_start, m_len = segs[e]
        cnt = -(-m_len // MCHUNK)
        s = 2 * (-(-m_len // (2 * cnt)))  # even, balanced
        sizes = [s] * (cnt - 1) + [m_len - s * (cnt - 1)]
        m0 = m_start
        for mjw in sizes:
            mjw_mm = mjw + (mjw & 1)
            chunks.append((m0, mjw, mjw_mm))
            m0 += mjw
        elist.append((e, cnt))
    return chunks, elist


def _build_program(segs):
    from concourse import bacc
    import concourse.mybir as mybir
    import concourse.tile as tile

    f32 = mybir.dt.float32
    bf16 = mybir.dt.bfloat16

    chunks, elist = _chunks_of(segs)
    nch = len(chunks)

    nc = bacc.Bacc(name="grouped_gemm")
    a_p = nc.declare_dram_parameter("a", [nch, P, KC, MCHUNK], bf16, isOutput=False)
    w_p = nc.declare_dram_parameter("w", [E, P, NCH_N, KC, P], bf16, isOutput=False)
    o_p = nc.declare_dram_parameter("o", [nch, P, NCH_N, MCHUNK], bf16, isOutput=True)

    with (
        tile.TileContext(nc) as tc,
        tc.tile_pool(name="wp", bufs=3) as wp,
        tc.tile_pool(name="apool", bufs=4) as apool,
        tc.tile_pool(name="spool", bufs=3) as spool,
        tc.tile_pool(name="pspool", bufs=8, space="PSUM") as pspool,
    ):
        ci = 0
        first = True
        for e, count in elist:
            w_t = wp.tile([P, NCH_N, KC, P], bf16, tag="w")
            if first:
                # Pipeline fill: the first expert's weights stream per
                # 128-column chunk across the sync and scalar rings (scalar
                # carries nothing else at t=0), so the ch=0 accumulation
                # group starts after ~0.25 MB instead of ~3 MB.
                for ch in range(NCH_N):
                    ring = nc.sync if ch % 2 == 0 else nc.scalar
                    ring.dma_start(w_t[:, ch], w_p[e, :, ch])
            else:
                nc.sync.dma_start(w_t[:], w_p[e])
            for _ in range(count):
                _, mjw, mjw_mm = chunks[ci]
                a_t = apool.tile([P, KC, MCHUNK], bf16, tag="a")
                if first:
                    # First chunk's a per kc-piece: the (ch0, kc0) matmul
                    # needs only the first piece.
                    for kc in range(KC):
                        nc.gpsimd.dma_start(
                            a_t[:, kc, :mjw_mm], a_p[ci, :, kc, :mjw_mm]
                        )
                    first = False
                else:
                    nc.gpsimd.dma_start(a_t[:], a_p[ci])
                st = spool.tile([P, NCH_N, MCHUNK], bf16, tag="st")
                for ch in range(NCH_N):
                    ps = pspool.tile([P, MCHUNK], f32, tag="ps")
                    for kc in range(KC):
                        nc.tensor.matmul(
                            ps[:, :mjw_mm],
                            w_t[:, ch, kc, :],
                            a_t[:, kc, :mjw_mm],
                            start=(kc == 0),
                            stop=(kc == KC - 1),
                        )
                    nc.vector.tensor_copy(st[:, ch, :mjw], ps[:, :mjw])
                    s_ring = nc.scalar if ch % 2 == 0 else nc.gpsimd
                    s_ring.dma_start(o_p[ci, :, ch, :mjw], st[:, ch, :mjw])
                ci += 1

    nc.finalize()
    return nc


def _get_program(segs):
    nc = _cache.get(segs)
    if nc is None:
        nc = _build_program(segs)
        _cache[segs] = nc
    return nc


def kernel(a, b, scale_a, scale_b, seg_indptr, batch_size, _want_trace=False):
    from concourse.bass_utils import run_bass_kernel_spmd

    a = np.asarray(a, dtype=np.float32)
    b = np.asarray(b, dtype=np.float32)
    scale_a = np.asarray(scale_a, dtype=np.float32).reshape(M, 1)
    scale_b = np.asarray(scale_b, dtype=np.float32).reshape(E, 1)
    seg = np.asarray(seg_indptr).astype(np.int64)

    segs = []
    row_scale = np.empty((M, 1), dtype=np.float32)
    for e in range(E):
        s, t = int(seg[e]), int(seg[e + 1])
        s, t = max(0, min(s, M)), max(0, min(t, M))
        segs.append((s, max(0, t - s)))
        if t > s:
            row_scale[s:t] = scale_b[e, 0]
    segs = tuple(segs)
    row_scale *= scale_a

    chunks, _counts = _chunks_of(segs)
    nch = len(chunks)
    nc = _get_program(segs)

    a_scaled = (a * row_scale).astype(BF16)  # [M, K]
    # Pack a chunks per K-half: a_pk[h][ci, p, kc, m]
    a_pk = [np.zeros((nch, P, KC, MCHUNK), dtype=BF16) for _ in range(2)]
    for ci, (m0, mjw, _mm) in enumerate(chunks):
        blk = a_scaled[m0 : m0 + mjw]  # [mjw, K]
        # [mjw, 2, 8, 128] -> (h, p, kc, m)
        blk4 = blk.reshape(mjw, 2, KC, P).transpose(1, 3, 2, 0)
        a_pk[0][ci, :, :, :mjw] = blk4[0]
        a_pk[1][ci, :, :, :mjw] = blk4[1]

    # Pack weights per core: w[e, p, kc, n] = b[e, g*1408+n, h*1024+kc*128+p]
    b16 = b.astype(BF16)
    in_maps = []
    for c in range(NCORES):
        g, h = c // 2, c % 2
        bw = b16[:, g * NSLICE : (g + 1) * NSLICE, h * KHALF : (h + 1) * KHALF]
        # [E, (ch,n128), (kc,p)] -> [E, p, ch, kc, n128]
        w_c = np.ascontiguousarray(
            bw.reshape(E, NCH_N, P, KC, P).transpose(0, 4, 1, 3, 2)
        )
        in_maps.append({"a": a_pk[h], "w": w_c})

    res = run_bass_kernel_spmd(
        nc, in_maps, list(range(NCORES)), trace=_want_trace
    )

    out = np.empty((M, N), dtype=np.float32)
    for g in range(NGROUPS):
        o_sum = res.results[2 * g]["o"].astype(np.float32) + res.results[
            2 * g + 1
        ]["o"].astype(np.float32)
        for ci, (m0, mjw, _mm) in enumerate(chunks):
            # [p, ch, m] -> [m, ch, p] -> [mjw, 1408]
            out[m0 : m0 + mjw, g * NSLICE : (g + 1) * NSLICE] = (
                o_sum[ci, :, :, :mjw].transpose(2, 1, 0).reshape(mjw, NSLICE)
            )
    if _want_trace:
        return out, res
    return out
